# revision 47
# baseline (speedup 1.0000x reference)
"""Causal multi-head attention (B=2, H=12, T=2048, D=64) on 8 Trainium2 NeuronCores.

Sharding: the 24 (batch, head) pairs are split 3-per-core across 8 cores.
Per head the device kernel computes, in transposed-score layout:

    S^T[kv, q] = K @ Q^T            (PE, fp16 in / fp32 PSUM out; the score
                                     matmuls alternate between the two 64-row
                                     halves of the PE array — K/Q are loaded
                                     duplicated on both partition halves so two
                                     matmuls run concurrently via PE row tiling)
    P^T        = exp(S^T * 1/8)     split column-wise between ACT (table exp)
                                     and DVE (Schraudolph bit-trick exp:
                                     int16(a*S+b) bitcast to fp16), both
                                     reading fp32 PSUM and writing fp16 SBUF
    diag block masked in-place on GPSIMD (affine_select, keep q >= kv)
    O'^T[65, 512] = [V | ones] @ P^T  per 4-q-tile group, j-merged matmuls
                                     (row 64 = softmax denominators)
    O'^T DMA'd straight from PSUM to HBM; the final divide by the
    denominator row and the [65, T] -> [T, 64] transpose happen on host.

The 136 causal (q-tile, kv-block) score blocks of each head form one linear
stream, chunked into PSUM megas of 12 blocks; exp is issued once per mega per
engine. The chunk stream is software-pipelined one chunk ahead of the PV
consumers and runs continuously across the 3 heads.

`repeat` > 1 wraps the whole body in a hardware For_i loop (timing aid only).

Self-contained: only imports numpy + the installed concourse/bass stack.
"""

import os
import numpy as np

B, H, T, D = 2, 12, 2048, 64
NCORES = 8
HPC = (B * H) // NCORES      # heads per core = 3
NQT = T // 128               # 16 q tiles of 128 rows
MEGA_BLKS = 12               # kv blocks per PSUM score tile (12*128 cols = 3 banks)
SCALE = 1.0 / 8.0            # 1/sqrt(D)

_cache = {}


def build_program(
    chunk_pattern=(16, 12),  # alternating chunk sizes (PSUM bank budget 4+3)
    io_bufs=3,
    pt_bufs=2,
    oq_bufs=2,
    tile2=True,          # PE row-tiling for the D=64 score matmuls
    act_frac=0.53,       # fraction of exp columns on ACT (rest on DVE)
    sch_bias=-26.0,      # Schraudolph bias correction (in fp16 ulp units)
    repeat=1,
    ablate=(),
):
    import concourse.bacc as bacc
    import concourse.mybir as mybir
    import concourse.tile as tile

    f16 = mybir.dt.float16
    f32 = mybir.dt.float32
    i16 = mybir.dt.int16
    Exp = mybir.ActivationFunctionType.Exp
    Mult = mybir.AluOpType.mult
    Add = mybir.AluOpType.add

    sch_a = float((2.0 ** 10) * np.log2(np.e) * SCALE)
    sch_b = float(15360.0 + sch_bias)

    nc = bacc.Bacc(None)
    qT_d = nc.dram_tensor("qT", [HPC, D, T], f16, kind="ExternalInput")
    kT_d = nc.dram_tensor("kT", [HPC, D, T], f16, kind="ExternalInput")
    v_d = nc.dram_tensor("v", [HPC, 128, T // 128, D + 1], f16, kind="ExternalInput")
    o_d = nc.dram_tensor("out", [HPC, D + 1, T], f32, kind="ExternalOutput")

    # j-major: all q-tiles for one kv-block are contiguous, so score matmuls
    # sharing the same stationary K-slice merge into wide ones, and the PV
    # moving operand for one kv-block spans up to 4 q-tiles contiguously
    blocks = [(i, j) for j in range(NQT) for i in range(j, NQT)]
    nblk = len(blocks)                      # 136
    off = {bl: 128 * n for n, bl in enumerate(blocks)}
    # uniform 8-block chunks over a 3-pool PSUM rotation (2 banks each,
    # leaving 2 banks for double-buffered PV accumulators): the score
    # matmuls reusing a pool are 3 chunks past the exp that frees it, so
    # the strict-FIFO PE never waits on an exp in steady state
    sizes = [8] * 16 + [4, 4]
    assert sum(sizes) == nblk
    chunks = []
    c0 = 0
    for size in sizes:
        chunks.append(blocks[c0:c0 + size])
        c0 += size
    nch = len(chunks)
    # group g (q-tiles 4g..4g+3) PV fires in two phases: the off-diagonal
    # phase (j < 4g) once block (4g+3, 4g-1) is exp'd, and the diagonal
    # phase (j >= 4g) once block (4g+3, 4g+3) is exp'd (no early phase for
    # g=0). Each phase is delayed one chunk past its gate so the PE (a
    # strict FIFO) never queues a matmul whose exp/mask gate is still
    # pending -- that would block the next chunk's score matmuls behind it.
    done_chunk = {}
    for ci, ch in enumerate(chunks):
        for (i, j) in ch:
            done_chunk[(i, j)] = ci
    # pv agenda keyed by GLOBAL stream position (h * nch + ci): events spill
    # across head boundaries instead of piling up at a head's end. Off-
    # diagonal PV work is dribbled in sub-batches of <=4 matmuls so the PE
    # FIFO never holds a long gated burst, and every event sits >=2 chunks
    # past the exp that produced its inputs (so its gates are long settled
    # by the time the strict-FIFO PE reaches it).
    pv_agenda = []   # ordered (position, h, g, phase, jlist)
    for h in range(HPC):
        for g in range(NQT // 4):
            late_pos = h * nch + done_chunk[(4 * g + 3, 4 * g + 3)] + 2
            if g > 0:
                early_pos = h * nch + done_chunk[(4 * g + 3, 4 * g - 1)] + 2
                js = list(range(4 * g))
                for k, j0 in enumerate(range(0, len(js), 4)):
                    pv_agenda.append(
                        (min(early_pos + k, late_pos), h, g, "early",
                         js[j0:j0 + 4])
                    )
            pv_agenda.append((late_pos, h, g, "late", None))
    pv_events = {}
    for (pos, h, g, phase, jlist) in pv_agenda:   # stable per-(h,g) order
        pv_events.setdefault(pos, []).append((h, g, phase, jlist))
    # diag-block masks fire right after the chunk that exp'd them (Pool is
    # otherwise idle, and this keeps the mask off the PV critical path)
    mask_chunk = {}
    for i in range(NQT):
        mask_chunk.setdefault(done_chunk[(i, i)], []).append(i)

    with tile.TileContext(nc) as tc:
        with (
            tc.tile_pool(name="consts", bufs=1) as consts,
            tc.tile_pool(name="qk", bufs=io_bufs) as qk,
            tc.tile_pool(name="vpool", bufs=io_bufs) as vpool,
            tc.tile_pool(name="ptpool", bufs=pt_bufs) as ptpool,
            tc.tile_pool(name="odrain", bufs=2) as odrain,
            tc.tile_pool(name="smega", bufs=1, space="PSUM") as smega,
            tc.tile_pool(name="smegb", bufs=1, space="PSUM") as smegb,
            tc.tile_pool(name="smegc", bufs=1, space="PSUM") as smegc,
            tc.tile_pool(name="oqp", bufs=oq_bufs, space="PSUM") as oqp,
        ):
            warm = consts.tile([128, 2], f32)
            nc.gpsimd.memset(warm[:], 0.0)
            wmm = consts.tile([128, 640], f16)
            nc.gpsimd.memset(wmm[:], 0.0)

            def emit_body():
                heads = {}
                state = {"bank": 0, "oq": {}, "exp": 0}

                def emit_loads(h):
                    qt = qk.tile([128, T], f16, tag="qt")
                    kt = qk.tile([128, T], f16, tag="kt")
                    # load K/Q duplicated on both partition halves so the
                    # two 64-row PE bands can each run score matmuls; for the
                    # first head, order the descriptors so chunk 0 (which
                    # runs entirely on band A) is gated only by the first two
                    if h == 0:
                        # split across the two DGE queues (SP + ACT) so the
                        # first chunk is gated by one descriptor per queue;
                        # ACT is idle here anyway
                        nc.scalar.dma_start(
                            qt[0:D, 0:1024], qT_d[h, :, 0:1024]
                        )
                        nc.scalar.dma_start(
                            qt[0:D, 1024:T], qT_d[h, :, 1024:T]
                        )
                        nc.sync.dma_start(kt[0:D, 0:384], kT_d[h, :, 0:384])
                        if tile2:
                            nc.sync.dma_start(
                                kt[D:2 * D, 0:384], kT_d[h, :, 0:384]
                            )
                            nc.scalar.dma_start(qt[D:2 * D, :], qT_d[h])
                        nc.sync.dma_start(kt[0:D, 384:T], kT_d[h, :, 384:T])
                    else:
                        nc.sync.dma_start(kt[0:D, :], kT_d[h])
                        nc.sync.dma_start(qt[0:D, :], qT_d[h])
                    if tile2:
                        if h == 0:
                            nc.sync.dma_start(
                                kt[D:2 * D, 384:T], kT_d[h, :, 384:T]
                            )
                        else:
                            nc.sync.dma_start(kt[D:2 * D, :], kT_d[h])
                            nc.sync.dma_start(qt[D:2 * D, :], qT_d[h])
                    vp = vpool.tile([128, NQT, D + 1], f16)
                    nc.sync.dma_start(vp[:], v_d[h])
                    pt = ptpool.tile([128, nblk * 128], f16, tag="pt")
                    heads[h] = {"qt": qt, "kt": kt, "vp": vp, "pt": pt}

                def emit_chunk(h, ci):
                    hd = heads[h]
                    ch = chunks[ci]
                    ncols = len(ch) * 128
                    pool = (smega, smegb, smegc)[ci % 3]
                    sm = pool.tile([128, ncols], f32, tag="sm")
                    # merge runs of consecutive-(i) blocks sharing j into one
                    # wide matmul (N <= 512 per PSUM-bank rule); alternate the
                    # two PE row bands per PSUM *bank* so matmuls on adjacent
                    # banks execute concurrently (two row tiles must never
                    # write the same PSUM bank simultaneously)
                    idx = 0
                    while idx < len(ch):
                        i0, j0 = ch[idx]
                        run = 1
                        maxrun = 4 - (idx % 4)  # stay within one PSUM bank
                        while (
                            run < maxrun
                            and idx + run < len(ch)
                            and ch[idx + run] == (i0 + run, j0)
                        ):
                            run += 1
                        if tile2 and not (h == 0 and ci <= 6):
                            hb = 64 * ((state["bank"] + idx // 4) % 2)
                        else:
                            hb = 0  # first chunks on band A only (fast start)
                        nc.tensor.matmul(
                            sm[:, idx * 128:(idx + run) * 128],
                            hd["kt"][hb:hb + D, j0 * 128:(j0 + 1) * 128],
                            hd["qt"][hb:hb + D, i0 * 128:(i0 + run) * 128],
                        )
                        idx += run
                    state["bank"] += (len(ch) + 3) // 4
                    # exp: whole chunk on ONE engine, alternating ACT/DVE
                    # (one per-call overhead per chunk instead of two; the
                    # 3-chunk PSUM rotation slack absorbs the longer
                    # single-engine latency)
                    pt0 = off[ch[0]]
                    use_act = ci % 2 == 0 or ci == nch - 1
                    if "dve" in ablate:
                        use_act = True
                    elif "act" in ablate:
                        use_act = False
                    tail = h == HPC - 1 and ci >= nch - 3 and not ablate
                    if tail:
                        # final chunks of the final head sit on the kernel
                        # tail: split them across both engines to halve the
                        # exp latency before the last masks/PV/drain chain
                        ca = ncols // 2
                        nc.scalar.activation(
                            hd["pt"][:, pt0:pt0 + ca], sm[:, 0:ca], Exp,
                            scale=SCALE,
                        )
                        nc.vector.tensor_scalar(
                            out=hd["pt"][:, pt0 + ca:pt0 + ncols].bitcast(i16),
                            in0=sm[:, ca:ncols],
                            scalar1=sch_a,
                            scalar2=sch_b,
                            op0=Mult,
                            op1=Add,
                        )
                    elif use_act:
                        nc.scalar.activation(
                            hd["pt"][:, pt0:pt0 + ncols], sm[:], Exp, scale=SCALE
                        )
                    else:
                        nc.vector.tensor_scalar(
                            out=hd["pt"][:, pt0:pt0 + ncols].bitcast(i16),
                            in0=sm[:],
                            scalar1=sch_a,
                            scalar2=sch_b,
                            op0=Mult,
                            op1=Add,
                        )

                def emit_pv_early(h, g, jlist):
                    # off-diagonal PV phase sub-batch: j < 4g, full-width
                    hd = heads[h]
                    if jlist and jlist[0] == 0:
                        oq = oqp.tile([D + 1, 512], f32, tag="oq")
                        state["oq"][h] = oq
                    oq = state["oq"][h]
                    if "pv" in ablate:
                        jlist = [j for j in jlist if j == 0]
                    for j in jlist:
                        nc.tensor.matmul(
                            oq[:],
                            hd["vp"][:, j, :],
                            hd["pt"][:, off[(4 * g, j)]:off[(4 * g, j)] + 512],
                            start=(j == 0),
                            stop=False,
                            skip_group_check=True,
                        )

                def emit_masks(h, ci):
                    if "mask" in ablate:
                        return
                    pt = heads[h]["pt"]
                    for i in mask_chunk.get(ci, []):
                        pd = pt[:, off[(i, i)]:off[(i, i)] + 128]
                        nc.gpsimd.affine_select(
                            out=pd, in_=pd,
                            compare_op=mybir.AluOpType.is_ge,
                            fill=0.0, base=0,
                            # keep where (q - kv) >= 0
                            pattern=[[1, 128]], channel_multiplier=-1,
                        )

                def emit_pv_late(h, g):
                    hd = heads[h]
                    pt = hd["pt"]
                    if g == 0:
                        oq = oqp.tile([D + 1, 512], f32, tag="oq")
                    else:
                        oq = state["oq"].pop(h)
                    jmax = 4 * g + 3
                    final = h == HPC - 1 and g == NQT // 4 - 1
                    od = odrain.tile([D + 1, 512], f32)
                    for j in range(4 * g, jmax + 1) if "pv" not in ablate else []:
                        w = (4 * g + 4 - j) * 128
                        nc.tensor.matmul(
                            oq[:, (j - 4 * g) * 128:512],
                            hd["vp"][:, j, :],
                            pt[:, off[(j, j)]:off[(j, j)] + w],
                            start=(g == 0 and j == 0),
                            stop=(j == jmax),
                            skip_group_check=True,
                        )
                        if final:
                            # column slice (j-4g) has received its last
                            # accumulation: drain it now, engines and DGE
                            # queues alternating, so the kernel tail is only
                            # the final slice's copy + store
                            c = (j - 4 * g) * 128
                            if j % 2 == 0:
                                nc.vector.tensor_copy(
                                    od[:, c:c + 128], oq[:, c:c + 128]
                                )
                            else:
                                nc.scalar.copy(
                                    od[:, c:c + 128], oq[:, c:c + 128]
                                )
                            qd = nc.sync if j % 2 == 0 else nc.scalar
                            qd.dma_start(
                                o_d[h, :, 4 * g * 128 + c:4 * g * 128 + c + 128],
                                od[:, c:c + 128],
                            )
                    if not final:
                        # alternate the PSUM drain between DVE and ACT to
                        # even out the two engines' aggregate load
                        if (h * 4 + g) % 2 == 0:
                            nc.vector.tensor_copy(od[:], oq[:])
                        else:
                            nc.scalar.copy(od[:], oq[:])
                        nc.sync.dma_start(
                            o_d[h, :, 4 * g * 128:4 * g * 128 + 512], od[:]
                        )

                # one software-pipelined stream over (head, chunk)
                stream = [(h, ci) for h in range(HPC) for ci in range(nch)]
                npre = min(io_bufs, HPC)
                # dummy matmuls: fill the initial DMA wait with PE work so
                # the HAM clock-gate is warm when chunk 0's matmuls start
                wps = oqp.tile([D + 1, 512], f32, tag="oq", name="wps")
                for wi in range(6):
                    nc.tensor.matmul(
                        wps[:], wmm[:, 0:D + 1], wmm[:, 128:640],
                        start=True, stop=True, skip_group_check=True,
                    )
                for h in range(npre):
                    emit_loads(h)
                    if h == 0:
                        # warm the ACT exp table AFTER the first loads'
                        # descriptors so it doesn't delay them on the ACT
                        # DGE queue; it still completes during chunk 0's
                        # score matmuls
                        nc.scalar.activation(warm[:, 1:2], warm[:, 0:1], Exp)
                for n, (h, ci) in enumerate(stream):
                    if n == 0:
                        emit_chunk(h, ci)
                    if n + 1 < len(stream):
                        h2, ci2 = stream[n + 1]
                        if ci2 == 0 and h2 >= npre:
                            emit_loads(h2)
                        emit_chunk(h2, ci2)
                    emit_masks(h, ci)
                    for (hh, g, phase, jlist) in pv_events.get(n, []):
                        if phase == "early":
                            emit_pv_early(hh, g, jlist)
                        else:
                            emit_pv_late(hh, g)
                # events past the end of the stream (final head's tail)
                nlast = len(stream) - 1
                for n in sorted(k for k in pv_events if k > nlast):
                    for (hh, g, phase, jlist) in pv_events[n]:
                        if phase == "early":
                            emit_pv_early(hh, g, jlist)
                        else:
                            emit_pv_late(hh, g)

            if repeat == 1:
                emit_body()
            else:
                import concourse.mybir as _mb
                engs = (
                    _mb.EngineType.PE,
                    _mb.EngineType.Activation,
                    _mb.EngineType.DVE,
                    _mb.EngineType.SP,
                    _mb.EngineType.Pool,
                )
                with tc.For_i(0, repeat, 1, hint_engines=engs):
                    emit_body()

    nc.compile()
    return nc


def _get_program():
    if "nc" not in _cache:
        os.environ.setdefault("MYCRO_LOCAL_CACHE", "1")
        _cache["nc"] = build_program()
    return _cache["nc"]


def kernel(q, k, v):
    from concourse.bass_utils import run_bass_kernel_spmd

    q = np.asarray(q).reshape(B * H, T, D).astype(np.float16)
    k = np.asarray(k).reshape(B * H, T, D).astype(np.float16)
    qT = np.ascontiguousarray(q.transpose(0, 2, 1))
    kT = np.ascontiguousarray(k.transpose(0, 2, 1))
    # pre-pack V as [head, partition, kv-block, D+1] with a baked-in ones
    # column, so the device load is one contiguous descriptor per partition
    v = np.asarray(v).reshape(B * H, T // 128, 128, D).astype(np.float16)
    vp = np.ones((B * H, 128, T // 128, D + 1), np.float16)
    vp[:, :, :, 0:D] = v.transpose(0, 2, 1, 3)

    nc = _get_program()
    in_maps = [
        {
            "qT": qT[c * HPC:(c + 1) * HPC],
            "kT": kT[c * HPC:(c + 1) * HPC],
            "v": vp[c * HPC:(c + 1) * HPC],
        }
        for c in range(NCORES)
    ]
    res = run_bass_kernel_spmd(nc, in_maps, list(range(NCORES)))
    kernel._last = res
    raw = np.concatenate(
        [np.asarray(res.results[c]["out"]) for c in range(NCORES)], axis=0
    )  # [B*H, 65, T]
    out = raw[:, 0:D, :] / raw[:, D:D + 1, :]
    out = np.ascontiguousarray(out.transpose(0, 2, 1), dtype=np.float32)
    return out.reshape(B, H, T, D)


# revision 48
# speedup vs baseline: 1.1621x; 1.1621x over previous
"""Causal multi-head attention (B=2, H=12, T=2048, D=64) on 8 Trainium2 NeuronCores.

Sharding: the 24 (batch, head) pairs are split 3-per-core across 8 cores.
Per head the device kernel computes, in transposed-score layout:

    S^T[kv, q] = K @ Q^T            (PE, fp16 in / fp32 PSUM out; the score
                                     matmuls alternate between the two 64-row
                                     halves of the PE array — K/Q are loaded
                                     duplicated on both partition halves so two
                                     matmuls run concurrently via PE row tiling)
    P^T        = exp(S^T * 1/8)     split column-wise between ACT (table exp)
                                     and DVE (Schraudolph bit-trick exp:
                                     int16(a*S+b) bitcast to fp16), both
                                     reading fp32 PSUM and writing fp16 SBUF
    diag block masked in-place on GPSIMD (affine_select, keep q >= kv)
    O'^T[65, 512] = [V | ones] @ P^T  per 4-q-tile group, j-merged matmuls
                                     (row 64 = softmax denominators)
    O'^T DMA'd straight from PSUM to HBM; the final divide by the
    denominator row and the [65, T] -> [T, 64] transpose happen on host.

The 136 causal (q-tile, kv-block) score blocks of each head form one linear
stream, chunked into PSUM megas of 12 blocks; exp is issued once per mega per
engine. The chunk stream is software-pipelined one chunk ahead of the PV
consumers and runs continuously across the 3 heads.

`repeat` > 1 wraps the whole body in a hardware For_i loop (timing aid only).

Self-contained: only imports numpy + the installed concourse/bass stack.
"""

import os
import numpy as np

B, H, T, D = 2, 12, 2048, 64
NCORES = 8
HPC = (B * H) // NCORES      # heads per core = 3
NQT = T // 128               # 16 q tiles of 128 rows
MEGA_BLKS = 12               # kv blocks per PSUM score tile (12*128 cols = 3 banks)
SCALE = 1.0 / 8.0            # 1/sqrt(D)

_cache = {}


def build_program(
    chunk_pattern=(16, 12),  # alternating chunk sizes (PSUM bank budget 4+3)
    io_bufs=3,
    pt_bufs=2,
    oq_bufs=2,
    tile2=True,          # PE row-tiling for the D=64 score matmuls
    act_frac=0.53,       # fraction of exp columns on ACT (rest on DVE)
    sch_bias=-26.0,      # Schraudolph bias correction (in fp16 ulp units)
    repeat=1,
    ablate=(),
):
    import concourse.bacc as bacc
    import concourse.mybir as mybir
    import concourse.tile as tile

    f16 = mybir.dt.float16
    f32 = mybir.dt.float32
    i16 = mybir.dt.int16
    Exp = mybir.ActivationFunctionType.Exp
    Mult = mybir.AluOpType.mult
    Add = mybir.AluOpType.add

    sch_a = float((2.0 ** 10) * np.log2(np.e) * SCALE)
    sch_b = float(15360.0 + sch_bias)

    nc = bacc.Bacc(None)
    qT_d = nc.dram_tensor("qT", [HPC, D, T], f16, kind="ExternalInput")
    kT_d = nc.dram_tensor("kT", [HPC, D, T], f16, kind="ExternalInput")
    v_d = nc.dram_tensor("v", [HPC, 128, T // 128, D + 1], f16, kind="ExternalInput")
    o_d = nc.dram_tensor("out", [HPC, D + 1, T], f32, kind="ExternalOutput")

    # j-major: all q-tiles for one kv-block are contiguous, so score matmuls
    # sharing the same stationary K-slice merge into wide ones, and the PV
    # moving operand for one kv-block spans up to 4 q-tiles contiguously
    blocks = [(i, j) for j in range(NQT) for i in range(j, NQT)]
    nblk = len(blocks)                      # 136
    off = {bl: 128 * n for n, bl in enumerate(blocks)}
    # uniform 8-block chunks over a 3-pool PSUM rotation (2 banks each,
    # leaving 2 banks for double-buffered PV accumulators): the score
    # matmuls reusing a pool are 3 chunks past the exp that frees it, so
    # the strict-FIFO PE never waits on an exp in steady state
    sizes = [8] * 16 + [4, 4]
    assert sum(sizes) == nblk
    chunks = []
    c0 = 0
    for size in sizes:
        chunks.append(blocks[c0:c0 + size])
        c0 += size
    nch = len(chunks)
    # group g (q-tiles 4g..4g+3) PV fires in two phases: the off-diagonal
    # phase (j < 4g) once block (4g+3, 4g-1) is exp'd, and the diagonal
    # phase (j >= 4g) once block (4g+3, 4g+3) is exp'd (no early phase for
    # g=0). Each phase is delayed one chunk past its gate so the PE (a
    # strict FIFO) never queues a matmul whose exp/mask gate is still
    # pending -- that would block the next chunk's score matmuls behind it.
    done_chunk = {}
    for ci, ch in enumerate(chunks):
        for (i, j) in ch:
            done_chunk[(i, j)] = ci
    # pv agenda keyed by GLOBAL stream position (h * nch + ci): events spill
    # across head boundaries instead of piling up at a head's end. Off-
    # diagonal PV work is dribbled in sub-batches of <=4 matmuls so the PE
    # FIFO never holds a long gated burst, and every event sits >=2 chunks
    # past the exp that produced its inputs (so its gates are long settled
    # by the time the strict-FIFO PE reaches it).
    pv_agenda = []   # ordered (position, h, g, phase, jlist)
    for h in range(HPC):
        for g in range(NQT // 4):
            late_pos = h * nch + done_chunk[(4 * g + 3, 4 * g + 3)] + 2
            if g > 0:
                early_pos = h * nch + done_chunk[(4 * g + 3, 4 * g - 1)] + 2
                js = list(range(4 * g))
                for k, j0 in enumerate(range(0, len(js), 4)):
                    pv_agenda.append(
                        (min(early_pos + k, late_pos), h, g, "early",
                         js[j0:j0 + 4])
                    )
            pv_agenda.append((late_pos, h, g, "late", None))
    pv_events = {}
    for (pos, h, g, phase, jlist) in pv_agenda:   # stable per-(h,g) order
        pv_events.setdefault(pos, []).append((h, g, phase, jlist))
    # diag-block masks fire right after the chunk that exp'd them (Pool is
    # otherwise idle, and this keeps the mask off the PV critical path)
    mask_chunk = {}
    for i in range(NQT):
        mask_chunk.setdefault(done_chunk[(i, i)], []).append(i)

    with tile.TileContext(nc) as tc:
        with (
            tc.tile_pool(name="consts", bufs=1) as consts,
            tc.tile_pool(name="qk", bufs=io_bufs) as qk,
            tc.tile_pool(name="vpool", bufs=io_bufs) as vpool,
            tc.tile_pool(name="ptpool", bufs=pt_bufs) as ptpool,
            tc.tile_pool(name="odrain", bufs=2) as odrain,
            tc.tile_pool(name="smega", bufs=1, space="PSUM") as smega,
            tc.tile_pool(name="smegb", bufs=1, space="PSUM") as smegb,
            tc.tile_pool(name="smegc", bufs=1, space="PSUM") as smegc,
            tc.tile_pool(name="oqp", bufs=oq_bufs, space="PSUM") as oqp,
        ):
            warm = consts.tile([128, 2], f32)
            nc.gpsimd.memset(warm[:], 0.0)
            wmm = consts.tile([128, 640], f16)
            nc.gpsimd.memset(wmm[:], 0.0)

            def emit_body():
                heads = {}
                state = {"bank": 0, "oq": {}, "exp": 0}

                def emit_loads(h):
                    qt = qk.tile([128, T], f16, tag="qt")
                    kt = qk.tile([128, T], f16, tag="kt")
                    # load K/Q duplicated on both partition halves so the
                    # two 64-row PE bands can each run score matmuls; for the
                    # first head, order the descriptors so chunk 0 (which
                    # runs entirely on band A) is gated only by the first two
                    if h == 0:
                        # split across the two DGE queues (SP + ACT) so the
                        # first chunk is gated by one descriptor per queue;
                        # ACT is idle here anyway
                        nc.scalar.dma_start(
                            qt[0:D, 0:1024], qT_d[h, :, 0:1024]
                        )
                        nc.scalar.dma_start(
                            qt[0:D, 1024:T], qT_d[h, :, 1024:T]
                        )
                        nc.sync.dma_start(kt[0:D, 0:384], kT_d[h, :, 0:384])
                        if tile2:
                            nc.sync.dma_start(
                                kt[D:2 * D, 0:384], kT_d[h, :, 0:384]
                            )
                            nc.scalar.dma_start(qt[D:2 * D, :], qT_d[h])
                        nc.sync.dma_start(kt[0:D, 384:T], kT_d[h, :, 384:T])
                    else:
                        nc.sync.dma_start(kt[0:D, :], kT_d[h])
                        nc.sync.dma_start(qt[0:D, :], qT_d[h])
                    if tile2:
                        if h == 0:
                            nc.sync.dma_start(
                                kt[D:2 * D, 384:T], kT_d[h, :, 384:T]
                            )
                        else:
                            nc.sync.dma_start(kt[D:2 * D, :], kT_d[h])
                            nc.sync.dma_start(qt[D:2 * D, :], qT_d[h])
                    vp = vpool.tile([128, NQT, D + 1], f16)
                    nc.sync.dma_start(vp[:], v_d[h])
                    pt = ptpool.tile([128, nblk * 128], f16, tag="pt")
                    heads[h] = {"qt": qt, "kt": kt, "vp": vp, "pt": pt}

                def emit_chunk(h, ci):
                    hd = heads[h]
                    ch = chunks[ci]
                    ncols = len(ch) * 128
                    pool = (smega, smegb, smegc)[ci % 3]
                    sm = pool.tile([128, ncols], f32, tag="sm")
                    # merge runs of consecutive-(i) blocks sharing j into one
                    # wide matmul (N <= 512 per PSUM-bank rule); alternate the
                    # two PE row bands per PSUM *bank* so matmuls on adjacent
                    # banks execute concurrently (two row tiles must never
                    # write the same PSUM bank simultaneously)
                    idx = 0
                    while idx < len(ch):
                        i0, j0 = ch[idx]
                        run = 1
                        maxrun = 4 - (idx % 4)  # stay within one PSUM bank
                        while (
                            run < maxrun
                            and idx + run < len(ch)
                            and ch[idx + run] == (i0 + run, j0)
                        ):
                            run += 1
                        if tile2 and not (h == 0 and ci <= 4):
                            hb = 64 * ((state["bank"] + idx // 4) % 2)
                        else:
                            hb = 0  # first chunks on band A only (fast start)
                        nc.tensor.matmul(
                            sm[:, idx * 128:(idx + run) * 128],
                            hd["kt"][hb:hb + D, j0 * 128:(j0 + 1) * 128],
                            hd["qt"][hb:hb + D, i0 * 128:(i0 + run) * 128],
                        )
                        idx += run
                    state["bank"] += (len(ch) + 3) // 4
                    # exp: whole chunk on ONE engine, alternating ACT/DVE
                    # (one per-call overhead per chunk instead of two; the
                    # 3-chunk PSUM rotation slack absorbs the longer
                    # single-engine latency)
                    pt0 = off[ch[0]]
                    use_act = ci % 2 == 0 or ci == nch - 1
                    if "dve" in ablate:
                        use_act = True
                    elif "act" in ablate:
                        use_act = False
                    if use_act:
                        nc.scalar.activation(
                            hd["pt"][:, pt0:pt0 + ncols], sm[:], Exp, scale=SCALE
                        )
                    else:
                        nc.vector.tensor_scalar(
                            out=hd["pt"][:, pt0:pt0 + ncols].bitcast(i16),
                            in0=sm[:],
                            scalar1=sch_a,
                            scalar2=sch_b,
                            op0=Mult,
                            op1=Add,
                        )

                def emit_pv_early(h, g, jlist):
                    # off-diagonal PV phase sub-batch: j < 4g, full-width
                    hd = heads[h]
                    if jlist and jlist[0] == 0:
                        oq = oqp.tile([D + 1, 512], f32, tag="oq")
                        state["oq"][h] = oq
                    oq = state["oq"][h]
                    if "pv" in ablate:
                        jlist = [j for j in jlist if j == 0]
                    for j in jlist:
                        nc.tensor.matmul(
                            oq[:],
                            hd["vp"][:, j, :],
                            hd["pt"][:, off[(4 * g, j)]:off[(4 * g, j)] + 512],
                            start=(j == 0),
                            stop=False,
                            skip_group_check=True,
                        )

                def emit_masks(h, ci):
                    if "mask" in ablate:
                        return
                    pt = heads[h]["pt"]
                    for i in mask_chunk.get(ci, []):
                        pd = pt[:, off[(i, i)]:off[(i, i)] + 128]
                        nc.gpsimd.affine_select(
                            out=pd, in_=pd,
                            compare_op=mybir.AluOpType.is_ge,
                            fill=0.0, base=0,
                            # keep where (q - kv) >= 0
                            pattern=[[1, 128]], channel_multiplier=-1,
                        )

                def emit_pv_late(h, g):
                    hd = heads[h]
                    pt = hd["pt"]
                    if g == 0:
                        oq = oqp.tile([D + 1, 512], f32, tag="oq")
                    else:
                        oq = state["oq"].pop(h)
                    jmax = 4 * g + 3
                    final = h == HPC - 1 and g == NQT // 4 - 1
                    od = odrain.tile([D + 1, 512], f32)
                    for j in range(4 * g, jmax + 1) if "pv" not in ablate else []:
                        w = (4 * g + 4 - j) * 128
                        nc.tensor.matmul(
                            oq[:, (j - 4 * g) * 128:512],
                            hd["vp"][:, j, :],
                            pt[:, off[(j, j)]:off[(j, j)] + w],
                            start=(g == 0 and j == 0),
                            stop=(j == jmax),
                            skip_group_check=True,
                        )
                        if final:
                            # column slice (j-4g) has received its last
                            # accumulation: drain it now, engines and DGE
                            # queues alternating, so the kernel tail is only
                            # the final slice's copy + store
                            c = (j - 4 * g) * 128
                            if j % 2 == 0:
                                nc.vector.tensor_copy(
                                    od[:, c:c + 128], oq[:, c:c + 128]
                                )
                            else:
                                nc.scalar.copy(
                                    od[:, c:c + 128], oq[:, c:c + 128]
                                )
                            qd = nc.sync if j % 2 == 0 else nc.scalar
                            qd.dma_start(
                                o_d[h, :, 4 * g * 128 + c:4 * g * 128 + c + 128],
                                od[:, c:c + 128],
                            )
                    if not final:
                        # alternate the PSUM drain between DVE and ACT to
                        # even out the two engines' aggregate load
                        if (h * 4 + g) % 2 == 0:
                            nc.vector.tensor_copy(od[:], oq[:])
                        else:
                            nc.scalar.copy(od[:], oq[:])
                        nc.sync.dma_start(
                            o_d[h, :, 4 * g * 128:4 * g * 128 + 512], od[:]
                        )

                # one software-pipelined stream over (head, chunk)
                stream = [(h, ci) for h in range(HPC) for ci in range(nch)]
                npre = min(io_bufs, HPC)
                # dummy matmuls: fill the initial DMA wait with PE work so
                # the HAM clock-gate is warm when chunk 0's matmuls start
                wps = oqp.tile([D + 1, 512], f32, tag="oq", name="wps")
                for wi in range(6):
                    nc.tensor.matmul(
                        wps[:], wmm[:, 0:D + 1], wmm[:, 128:640],
                        start=True, stop=True, skip_group_check=True,
                    )
                for h in range(npre):
                    emit_loads(h)
                    if h == 0:
                        # warm the ACT exp table AFTER the first loads'
                        # descriptors so it doesn't delay them on the ACT
                        # DGE queue; it still completes during chunk 0's
                        # score matmuls
                        nc.scalar.activation(warm[:, 1:2], warm[:, 0:1], Exp)
                for n, (h, ci) in enumerate(stream):
                    if n == 0:
                        emit_chunk(h, ci)
                    if n + 1 < len(stream):
                        h2, ci2 = stream[n + 1]
                        if ci2 == 0 and h2 >= npre:
                            emit_loads(h2)
                        emit_chunk(h2, ci2)
                    emit_masks(h, ci)
                    for (hh, g, phase, jlist) in pv_events.get(n, []):
                        if phase == "early":
                            emit_pv_early(hh, g, jlist)
                        else:
                            emit_pv_late(hh, g)
                # events past the end of the stream (final head's tail)
                nlast = len(stream) - 1
                for n in sorted(k for k in pv_events if k > nlast):
                    for (hh, g, phase, jlist) in pv_events[n]:
                        if phase == "early":
                            emit_pv_early(hh, g, jlist)
                        else:
                            emit_pv_late(hh, g)

            if repeat == 1:
                emit_body()
            else:
                import concourse.mybir as _mb
                engs = (
                    _mb.EngineType.PE,
                    _mb.EngineType.Activation,
                    _mb.EngineType.DVE,
                    _mb.EngineType.SP,
                    _mb.EngineType.Pool,
                )
                with tc.For_i(0, repeat, 1, hint_engines=engs):
                    emit_body()

    nc.compile()
    return nc


def _get_program():
    if "nc" not in _cache:
        os.environ.setdefault("MYCRO_LOCAL_CACHE", "1")
        _cache["nc"] = build_program()
    return _cache["nc"]


def kernel(q, k, v):
    from concourse.bass_utils import run_bass_kernel_spmd

    q = np.asarray(q).reshape(B * H, T, D).astype(np.float16)
    k = np.asarray(k).reshape(B * H, T, D).astype(np.float16)
    qT = np.ascontiguousarray(q.transpose(0, 2, 1))
    kT = np.ascontiguousarray(k.transpose(0, 2, 1))
    # pre-pack V as [head, partition, kv-block, D+1] with a baked-in ones
    # column, so the device load is one contiguous descriptor per partition
    v = np.asarray(v).reshape(B * H, T // 128, 128, D).astype(np.float16)
    vp = np.ones((B * H, 128, T // 128, D + 1), np.float16)
    vp[:, :, :, 0:D] = v.transpose(0, 2, 1, 3)

    nc = _get_program()
    in_maps = [
        {
            "qT": qT[c * HPC:(c + 1) * HPC],
            "kT": kT[c * HPC:(c + 1) * HPC],
            "v": vp[c * HPC:(c + 1) * HPC],
        }
        for c in range(NCORES)
    ]
    res = run_bass_kernel_spmd(nc, in_maps, list(range(NCORES)))
    kernel._last = res
    raw = np.concatenate(
        [np.asarray(res.results[c]["out"]) for c in range(NCORES)], axis=0
    )  # [B*H, 65, T]
    out = raw[:, 0:D, :] / raw[:, D:D + 1, :]
    out = np.ascontiguousarray(out.transpose(0, 2, 1), dtype=np.float32)
    return out.reshape(B, H, T, D)
